# revision 22
# baseline (speedup 1.0000x reference)
"""Trainium2 Bass kernel for nn_Autotuner_FFN (dense MLP, 8-core data parallel).

Strategy:
  * Host folds all embedding tables / 57 op-linears / log2-scalings and the
    LayerNorm mean-centerings into one effective first-layer weight matrix
    W1_eff [185, 1024] (+ centered biases). One-hot index encodings become
    extra GEMM rows. Per-element device work shrinks to:
        u = sign(x)*ln(|x|+1) on 57 of 185 feature rows,
        3 GEMMs (185->1024 -> 1024->1024 -> 1024->1),
        2 RMS-style norms (mean already folded) + relu (+g,be affine).
  * Device layout: activations transposed (hidden on partitions, batch on
    free dim). LN stats (sum of squares over hidden) via ones-vector matmul
    on TensorE; rsqrt broadcast back via a rank-1 outer-product matmul.
  * All matmul operands and the elementwise chain are fp16: matmuls stream
    1 cyc/col with FWL; DVE elementwise ops hit the 2x_1p packed mode.
  * Scalar engine uses only {Ln, Exp, Sign, Identity} — one activation
    table set (natural_log_exp_and_others), so zero mid-kernel table swaps.
    rsqrt(v) is computed as exp(-0.5*ln(v)) on [1,CH] stats rows.
  * scale+relu fused into one DVE scalar_tensor_tensor: R = max(H,0)*pv
    (valid because pv = rsqrt(var) > 0).
  * Batch 65536 is sharded 8192/core across 8 NeuronCores (pure DP).
"""
import numpy as np

import concourse.bass as bass
import concourse.tile as tile
from concourse import bacc, mybir
from concourse.bass_utils import run_bass_kernel_spmd

# The act-table insertion pass keeps a "current set" and greedily switches to
# the FIRST act_info.json set containing a missing function. With Ln and Exp
# both needed, that thrashes exp_and_others <-> natural_log (~2.7us per swap,
# 4 swaps per chunk). Every function this kernel uses (ln, exp, sign,
# identity, relu, copy, square) lives in natural_log_exp_and_others, so blank
# out every other set (preserving dict order = act_func_set_id indices) to
# pin the pass to the one set that covers the whole program.
_ONE_SET = "natural_log_exp_and_others"
_orig_get_tables = bacc.get_activation_tables


def _pinned_tables(arch):
    t = dict(_orig_get_tables(arch))
    if _ONE_SET not in t:  # unexpected act_info — keep stock behavior
        return t
    return {k: (v if k == _ONE_SET else set()) for k, v in t.items()}


bacc.get_activation_tables = _pinned_tables

AF = mybir.ActivationFunctionType
ALU = mybir.AluOpType
F32 = mybir.dt.float32
F16 = mybir.dt.float16
I16 = mybir.dt.int16

B = 65536
N_CORES = 8
B_CORE = B // N_CORES          # 8192
CH = 512                       # batch chunk (one PSUM bank wide)
NCH = B_CORE // CH             # 16
HID = 1024
MT = HID // 128                # 8 hidden m-tiles
KA, KC = 128, 57               # feature K tiles (125+3pad | 57 transformed)
EPS = 1e-5
LN2 = float(np.log(2.0))


# ---------------------------------------------------------------- host folds
def _fold_weights(inp):
    f8 = lambda x: np.asarray(x, np.float64)
    W1 = f8(inp["W1"]); b1 = f8(inp["b1"])
    emb_kc = f8(inp["emb_kc"]); emb_nl = f8(inp["emb_nl"])
    op_W = f8(inp["op_W"]); op_b = f8(inp["op_b"])
    emb_c = f8(inp["emb_contig"]); emb_s = f8(inp["emb_scalar"])
    emb_i = f8(inp["emb_indirect"])
    H = W1.shape[1]
    rows_A = []
    bias = b1.copy()
    rows_A.append(emb_kc @ W1[0:16])
    rows_A.append(emb_nl @ W1[16:32])
    W1_op = W1[32:944].reshape(57, 16, H)
    rows_A.append(np.einsum("ij,ijh->ih", op_W, W1_op))
    bias += np.einsum("ij,ijh->h", op_b, W1_op)
    rd_f2, rd_bool, rd_ss = [], [], []
    wd_f2, wd_bool, wd_ss = [], [], []
    for base, f2l, booll, ssl in ((947, rd_f2, rd_bool, rd_ss),
                                  (1027, wd_f2, wd_bool, wd_ss)):
        for d in range(4):
            Wd = W1[base + 20 * d: base + 20 * d + 20]
            f2l.append(Wd[0:2])
            ssl.append(Wd[2:8] / LN2)
            rows_b = []
            for e, sl in ((emb_c, slice(8, 12)), (emb_s, slice(12, 16)),
                          (emb_i, slice(16, 20))):
                rows_b.append((e[1] - e[0]) @ Wd[sl])
                bias += e[0] @ Wd[sl]
            booll.append(np.stack(rows_b))
    rows_A += [np.concatenate(rd_f2), np.concatenate(rd_bool),
               np.concatenate(wd_f2), np.concatenate(wd_bool),
               W1[1110:1112]]
    A = np.concatenate(rows_A)
    C = np.concatenate([W1[944:947] / LN2, W1[1107:1110] / LN2,
                        W1[1112:1115] / LN2,
                        np.concatenate(rd_ss), np.concatenate(wd_ss)])
    W1_eff = np.concatenate([A, np.zeros((3, H)), C])       # [185, H]
    W1c = W1_eff - W1_eff.mean(axis=1, keepdims=True)
    bc1 = bias - bias.mean()
    W2 = f8(inp["W2"]); b2 = f8(inp["b2"])
    W2c = W2 - W2.mean(axis=1, keepdims=True)
    bc2 = b2 - b2.mean()
    return (W1c.astype(np.float16), bc1.astype(np.float32),
            W2c.astype(np.float16), bc2.astype(np.float32))


def _build_xt(inp):
    Bn = inp["op_vec"].shape[0]
    kc = np.asarray(inp["kernel_category_idx"]).astype(np.int64)
    nl = np.asarray(inp["num_of_loops_idx"]).astype(np.int64)
    f = lambda k: np.asarray(inp[k], np.float32)
    XT = np.zeros((KA + KC, Bn), np.float16)
    XT[0:10] = (np.arange(10)[:, None] == kc[None, :])
    XT[10:26] = (np.arange(16)[:, None] == nl[None, :])
    XT[26:83] = f("op_vec").T
    XT[83:91] = f("read_dep_float")[:, :, 0:2].reshape(Bn, 8).T
    XT[91:103] = np.asarray(inp["read_dep_bools"]).reshape(Bn, 12).T
    XT[103:111] = f("write_dep_float")[:, :, 0:2].reshape(Bn, 8).T
    XT[111:123] = np.asarray(inp["write_dep_bools"]).reshape(Bn, 12).T
    XT[123:125] = f("rest_vec")[:, 3:5].T
    XT[128:131] = f("size_hints").T
    XT[131:137] = f("rest_vec")[:, [0, 1, 2, 5, 6, 7]].T
    XT[137:161] = f("read_dep_float")[:, :, 2:8].reshape(Bn, 24).T
    XT[161:185] = f("write_dep_float")[:, :, 2:8].reshape(Bn, 24).T
    return XT


def _pack128(v):
    """[1024] -> [128, 8] with v[m*128+p] at [p, m]."""
    return np.ascontiguousarray(np.asarray(v, np.float32).reshape(8, 128).T)


# ---------------------------------------------------------------- device prog
DEFAULT_CFG = dict(h_bufs=3, sq_bufs=4, r1_bufs=2, r2_bufs=2,
                   ps_mm_bufs=4, ps_st_bufs=2, ps_vec_bufs=2,
                   xin_bufs=3, xr_bufs=3, small_bufs=2, l2_pairs=2)


def build_program(simple_affine, loop_iters=None, cfg=None):
    """Build the per-core bass program, software-pipelined 3 chunks deep.

    Pipeline stages per chunk c (emitted stage-shifted so the PE stream of
    chunk c's L2 GEMMs overlaps chunk c+1's L1 and chunk c-1's L3):
      S0(c): DMA x + transform u = sign(x)*ln(|x|+1)
      S1(c): L1 matmuls + bias copy-out + per-m square + LN1 stats + rsqrt1
      S2(c): LN1 broadcast + scale+relu -> R1
      S3(c): L2 matmuls (m-pairs over k) + copy-out + square + stats + rsqrt2
      S4(c): LN2 broadcast + scale+relu -> R2
      S5(c): L3 matmuls + bias + DMA out
    """
    cfg = {**DEFAULT_CFG, **(cfg or {})}
    nc = bacc.Bacc("TRN2", target_bir_lowering=False, debug=False)
    xt = nc.dram_tensor("xt", [KA + KC, B_CORE], F16, kind="ExternalInput")
    w1 = nc.dram_tensor("w1", [KA + KC, HID], F16, kind="ExternalInput")
    w2 = nc.dram_tensor("w2", [HID, HID], F16, kind="ExternalInput")
    w3p = nc.dram_tensor("w3p", [128, MT], F16, kind="ExternalInput")
    bc1p = nc.dram_tensor("bc1p", [128, MT], F32, kind="ExternalInput")
    bc2p = nc.dram_tensor("bc2p", [128, MT], F32, kind="ExternalInput")
    g1p = nc.dram_tensor("g1p", [128, MT], F32, kind="ExternalInput")
    be1p = nc.dram_tensor("be1p", [128, MT], F32, kind="ExternalInput")
    g2p = nc.dram_tensor("g2p", [128, MT], F32, kind="ExternalInput")
    be2p = nc.dram_tensor("be2p", [128, MT], F32, kind="ExternalInput")
    b3t = nc.dram_tensor("b3t", [1, 1], F32, kind="ExternalInput")
    y = nc.dram_tensor("y", [1, B_CORE], F32, kind="ExternalOutput")

    from contextlib import ExitStack
    with tile.TileContext(nc) as tc, ExitStack() as ctx, \
            nc.allow_low_precision(reason="fp16 rounding is intentional"):
        const = ctx.enter_context(tc.tile_pool(name="const", bufs=1))
        xin = ctx.enter_context(tc.tile_pool(name="xin", bufs=cfg["xin_bufs"]))
        xr = ctx.enter_context(tc.tile_pool(name="xr", bufs=cfg["xr_bufs"]))
        bigH = ctx.enter_context(tc.tile_pool(name="bigH", bufs=cfg["h_bufs"]))
        sqp = ctx.enter_context(tc.tile_pool(name="sqp", bufs=cfg["sq_bufs"]))
        bigR1 = ctx.enter_context(tc.tile_pool(name="bigR1", bufs=cfg["r1_bufs"]))
        bigR2 = ctx.enter_context(tc.tile_pool(name="bigR2", bufs=cfg["r2_bufs"]))
        small = ctx.enter_context(tc.tile_pool(name="small", bufs=cfg["small_bufs"]))
        ps_mm = ctx.enter_context(tc.tile_pool(name="ps_mm", bufs=cfg["ps_mm_bufs"], space="PSUM"))
        ps_st = ctx.enter_context(tc.tile_pool(name="ps_st", bufs=cfg["ps_st_bufs"], space="PSUM"))
        ps_vec = ctx.enter_context(tc.tile_pool(name="ps_vec", bufs=cfg["ps_vec_bufs"], space="PSUM"))

        # ---- one-time setup: everything arrives in fp16, no staging copies
        # (w1 + the first chunk's x are queued before the 2MB of w2 tiles so
        # the first L1 matmul isn't stuck behind them on the DMA queue)
        w1a_r = const.tile([128, HID], F16, tag="w1a")
        nc.sync.dma_start(w1a_r[:], w1.ap()[0:128, :])
        w1c_r = const.tile([KC, HID], F16, tag="w1c")
        nc.sync.dma_start(w1c_r[:], w1.ap()[128:185, :])
        w2r = []
        for k in range(MT):
            t = const.tile([128, HID], F16, tag=f"w2r{k}")
            nc.sync.dma_start(t[:], w2.ap()[k * 128:(k + 1) * 128, :])
            w2r.append(t)
        w3p_r = const.tile([128, MT], F16, tag="w3p")
        nc.sync.dma_start(w3p_r[:], w3p.ap())
        w3s32 = const.tile([128, MT], F32, tag="w3s32")
        nc.vector.tensor_copy(w3s32[:], w3p_r[:])

        def load_small(name, dram):
            t = const.tile([128, MT], F32, tag=name)
            nc.sync.dma_start(t[:], dram.ap())
            return t
        bc1s = load_small("bc1s", bc1p); bc2s = load_small("bc2s", bc2p)
        g1s = load_small("g1s", g1p); be1s = load_small("be1s", be1p)
        g2s = load_small("g2s", g2p); be2s = load_small("be2s", be2p)
        b3s = const.tile([1, 1], F32, tag="b3s")
        nc.sync.dma_start(b3s[:], b3t.ap())
        ones_st = const.tile([128, 1], F32, tag="ones_st")
        nc.vector.memset(ones_st[:], 1.0)
        ones_col = const.tile([128, 1], F16, tag="ones_col")
        nc.vector.tensor_copy(ones_col[:], ones_st[:])
        ones_rst = const.tile([1, 128], F32, tag="ones_rst")
        nc.vector.memset(ones_rst[:], 1.0)
        ones_row = const.tile([1, 128], F16, tag="ones_row")
        nc.vector.tensor_copy(ones_row[:], ones_rst[:])
        eps_t = const.tile([1, 1], F32, tag="eps_t")
        nc.vector.memset(eps_t[:], EPS)

        def rsqrt_stats(pst, tag, dt=F16):
            """pst [1, CH] f32 = sum(H^2) over hidden.
            Returns [1, CH] = 1/sqrt(pst/HID + EPS) via exp(-0.5*ln(v))
            (keeps scalar engine inside one activation table set)."""
            tln = small.tile([1, CH], F32, tag=f"tln{tag}")
            nc.scalar.activation(tln[:], pst[:], AF.Ln,
                                 bias=eps_t[:], scale=1.0 / HID)
            rs = small.tile([1, CH], dt, tag=f"rs{tag}")
            nc.scalar.activation(rs[:], tln[:], AF.Exp, scale=-0.5)
            return rs

        def scale_relu(Hb, rs, g_s, be_s, out_pool, out_tag, tag):
            """R = relu(H * bcast(rs)) per m-tile. Returns f16 tile."""
            pv = ps_vec.tile([128, CH], F32, tag="pv")
            nc.tensor.matmul(pv[:], ones_row[:], rs[:], start=True, stop=True)
            pvs = small.tile([128, CH], F16, tag=f"pvs{tag}")
            nc.vector.tensor_copy(pvs[:], pv[:])
            Rb = out_pool.tile([128, MT * CH], F16, tag=out_tag)
            for m in range(MT):
                sl = slice(m * CH, (m + 1) * CH)
                nc.vector.tensor_mul(Hb[:, sl], Hb[:, sl], pvs[:])
                if simple_affine:
                    nc.vector.tensor_scalar(
                        out=Rb[:, sl], in0=Hb[:, sl],
                        scalar1=0.0, scalar2=None, op0=ALU.max)
                else:
                    nc.scalar.activation(Rb[:, sl], Hb[:, sl], AF.Relu,
                                         bias=be_s[:, m:m + 1],
                                         scale=g_s[:, m:m + 1])
            return Rb

        def make_state():
            return {}

        def s0_xprep(st, c):
            x1 = xin.tile([128, CH], F16, tag="x1")
            nc.sync.dma_start(x1[:], xt.ap()[0:128, c * CH:(c + 1) * CH])
            x2 = xin.tile([KC, CH], F16, tag="x2")
            nc.sync.dma_start(x2[:], xt.ap()[128:185, c * CH:(c + 1) * CH])
            # u = sign(x)*ln(|x|+1) on the 57 transformed rows
            xab = xr.tile([KC, CH], F16, tag="xab")
            nc.vector.tensor_scalar(
                out=xab[:].bitcast(I16),
                in0=x2[:].bitcast(I16),
                scalar1=0x7FFF, scalar2=None, op0=ALU.bitwise_and)
            xln = xr.tile([KC, CH], F16, tag="xln")
            nc.scalar.activation(xln[:], xab[:], AF.Ln, bias=1.0)
            xsg = xr.tile([KC, CH], F16, tag="xsg")
            nc.scalar.activation(xsg[:], x2[:], AF.Sign)
            x2r = xr.tile([KC, CH], F16, tag="x2r")
            nc.vector.tensor_mul(x2r[:], xsg[:], xln[:])
            st[("x", c)] = (x1, x2r)

        def acc_tree_add(accA, accB, t, m, tag):
            """Accumulate per-m tiles into accA (m 0-3, DVE) and accB
            (m 4-7, GPSIMD) as two parallel chains; caller joins at m==7
            with a final DVE add. t=None means the value was already
            written directly into the chain head by the producer."""
            if t is not None:
                if m < 4:
                    nc.vector.tensor_add(accA[:], accA[:], t[:])
                else:
                    nc.gpsimd.tensor_add(accB[:], accB[:], t[:])
            if m == MT - 1:
                nc.vector.tensor_add(accA[:], accA[:], accB[:])

        def s1_l1(st, c):
            x1, x2r = st.pop(("x", c))
            H1 = bigH.tile([128, MT * CH], F16, tag="H1")
            pst = ps_st.tile([1, CH], F32, tag="pst")
            for m in range(MT):
                p1 = ps_mm.tile([128, CH], F32, tag="pmm")
                nc.tensor.matmul(p1[:], w1a_r[:, m * 128:(m + 1) * 128],
                                 x1[:], start=True, stop=False)
                nc.tensor.matmul(p1[:], w1c_r[:, m * 128:(m + 1) * 128],
                                 x2r[:], start=False, stop=True)
                sl = slice(m * CH, (m + 1) * CH)
                nc.scalar.activation(H1[:, sl], p1[:],
                                     AF.Identity, bias=bc1s[:, m:m + 1])
                sq = sqp.tile([128, CH], F16, tag="sqa")
                nc.vector.tensor_mul(sq[:], H1[:, sl], H1[:, sl])
                nc.tensor.matmul(pst[:], ones_col[:], sq[:],
                                 start=(m == 0), stop=(m == MT - 1))
            st[("h1", c)] = H1
            st[("rs1", c)] = rsqrt_stats(pst, "a")

        def s2_ln1(st, c):
            H1 = st.pop(("h1", c))
            rs = st.pop(("rs1", c))
            st[("r1", c)] = scale_relu(H1, rs, g1s, be1s, bigR1, "R1", "a")

        def s3_l2(st, c):
            """L2 + LN2 stats. For the simple-affine case, the LN2 rescale
            is per-column, so it commutes past relu AND the (linear) L3
            contraction: y = rs2 * (w3.T relu(H2)) + b3. Q2 = relu(H2) is
            produced here per m-tile; no broadcast matmul / pv2 needed."""
            R1 = st.pop(("r1", c))
            H2 = bigH.tile([128, MT * CH], F16, tag="H2")
            pst = ps_st.tile([1, CH], F32, tag="pst")
            Q2 = bigR2.tile([128, MT * CH], F16, tag="R2")
            npair = cfg["l2_pairs"]
            for mp in range(0, MT, npair):
                ms = range(mp, mp + npair)
                p2 = {}
                for m in ms:
                    p2m = ps_mm.tile([128, CH], F32, tag="pmm")
                    p2[m] = p2m
                for k in range(MT):
                    for m in ms:
                        nc.tensor.matmul(
                            p2[m][:], w2r[k][:, m * 128:(m + 1) * 128],
                            R1[:, k * CH:(k + 1) * CH],
                            start=(k == 0), stop=(k == MT - 1))
                for m in ms:
                    sl = slice(m * CH, (m + 1) * CH)
                    nc.scalar.activation(H2[:, sl], p2[m][:],
                                         AF.Identity, bias=bc2s[:, m:m + 1])
                    sq = sqp.tile([128, CH], F16, tag="sqb")
                    nc.vector.tensor_mul(sq[:], H2[:, sl], H2[:, sl])
                    nc.tensor.matmul(pst[:], ones_col[:], sq[:],
                                     start=(m == 0), stop=(m == MT - 1))
                    if simple_affine:
                        nc.vector.tensor_scalar(
                            out=Q2[:, sl], in0=H2[:, sl],
                            scalar1=0.0, scalar2=None, op0=ALU.max)
            st[("h2", c)] = H2
            st[("q2", c)] = Q2
            st[("rs2", c)] = rsqrt_stats(pst, "b", dt=F32)

        def s5_l3(st, c):
            H2 = st.pop(("h2", c))
            rs2 = st.pop(("rs2", c))
            Q2 = st.pop(("q2", c))
            p3 = ps_st.tile([1, CH], F32, tag="pst")
            if simple_affine:
                for k in range(MT):
                    nc.tensor.matmul(p3[:], w3p_r[:, k:k + 1],
                                     Q2[:, k * CH:(k + 1) * CH],
                                     start=(k == 0), stop=(k == MT - 1))
            else:
                rs16 = small.tile([1, CH], F16, tag="rs2f16")
                nc.vector.tensor_copy(rs16[:], rs2[:])
                R2 = scale_relu(H2, rs16, g2s, be2s, bigR2, "R2x", "b")
                for k in range(MT):
                    nc.tensor.matmul(p3[:], w3p_r[:, k:k + 1],
                                     R2[:, k * CH:(k + 1) * CH],
                                     start=(k == 0), stop=(k == MT - 1))
            osb = small.tile([1, CH], F32, tag="osb")
            if simple_affine:
                yv = small.tile([1, CH], F32, tag="yv")
                nc.vector.tensor_mul(yv[:], p3[:], rs2[:])
                nc.scalar.activation(osb[:], yv[:], AF.Identity, bias=b3s[:])
            else:
                nc.scalar.activation(osb[:], p3[:], AF.Identity, bias=b3s[:])
            nc.sync.dma_start(y.ap()[0:1, c * CH:(c + 1) * CH], osb[:])

        def pipeline(st=None):
            if st is None:
                st = make_state()
                s0_xprep(st, 0)
            for i in range(NCH + 1):
                if i + 1 < NCH:
                    s0_xprep(st, i + 1)
                if i < NCH:
                    s1_l1(st, i)
                if 0 <= i - 1 < NCH:
                    s2_ln1(st, i - 1)
                    s3_l2(st, i - 1)
                    s5_l3(st, i - 1)

        if loop_iters is None:
            st0 = make_state()
            with tc.high_priority():
                s0_xprep(st0, 0)
            pipeline(st0)
        else:
            hint = tuple(
                mybir.EngineType(e) for e in
                ("PE", "Activation", "DVE", "SP", "Pool")
            ) if cfg.get("loop_hints") else ()
            with tc.For_i(0, loop_iters, 1,
                          staggered_reset=cfg.get("loop_stagger", False),
                          hint_engines=hint):
                pipeline()
    nc.compile()
    return nc


# ---------------------------------------------------------------- entry point
_CACHE = {}


BEST_CFG = dict()


def _get_program(simple_affine):
    key = ("prog", simple_affine)
    if key not in _CACHE:
        _CACHE[key] = build_program(simple_affine, cfg=BEST_CFG)
    return _CACHE[key]


def make_in_maps(inputs):
    inp = {k: np.asarray(v) for k, v in inputs.items()}
    W1c, bc1, W2c, bc2 = _fold_weights(inp)
    XT = _build_xt(inp)
    g1 = np.asarray(inp["g1"], np.float32); be1 = np.asarray(inp["be1"], np.float32)
    g2 = np.asarray(inp["g2"], np.float32); be2 = np.asarray(inp["be2"], np.float32)
    simple_affine = bool(
        np.all(g1 == 1.0) and np.all(g2 == 1.0)
        and np.all(be1 == 0.0) and np.all(be2 == 0.0))
    W3 = np.asarray(inp["W3"], np.float32)
    b3 = np.asarray(inp["b3"], np.float32)
    shared = {
        "w1": W1c, "w2": W2c,
        "w3p": _pack128(W3[:, 0]).astype(np.float16),
        "bc1p": _pack128(bc1), "bc2p": _pack128(bc2),
        "g1p": _pack128(g1), "be1p": _pack128(be1),
        "g2p": _pack128(g2), "be2p": _pack128(be2),
        "b3t": b3.reshape(1, 1),
    }
    in_maps = []
    for c in range(N_CORES):
        m = dict(shared)
        m["xt"] = np.ascontiguousarray(XT[:, c * B_CORE:(c + 1) * B_CORE])
        in_maps.append(m)
    return in_maps, simple_affine


def kernel(**inputs) -> np.ndarray:
    in_maps, simple_affine = make_in_maps(inputs)
    nc = _get_program(simple_affine)
    res = run_bass_kernel_spmd(nc, in_maps, core_ids=list(range(N_CORES)))
    y = np.concatenate([r["y"][0] for r in res.results])
    return y.reshape(B, 1).astype(np.float32)


if __name__ == "__main__":
    import jax
    import reference
    cpu = jax.devices("cpu")[0]
    with jax.default_device(cpu):
        inp = reference.setup_inputs()
        ref = np.asarray(reference.reference(**inp))
    out = kernel(**{k: np.asarray(v) for k, v in inp.items()})
    err = np.abs(out - ref)
    scale = np.abs(ref).max()
    print("max_abs", err.max(), "rel(vs scale)", err.max() / scale,
          "mean_rel", (err / (np.abs(ref) + 1e-6)).mean())


# revision 24
# speedup vs baseline: 1.0320x; 1.0320x over previous
"""Trainium2 Bass kernel for nn_Autotuner_FFN (dense MLP, 8-core data parallel).

Strategy:
  * Host folds all embedding tables / 57 op-linears / log2-scalings and the
    LayerNorm mean-centerings into one effective first-layer weight matrix
    W1_eff [185, 1024] (+ centered biases). One-hot index encodings become
    extra GEMM rows. Per-element device work shrinks to:
        u = sign(x)*ln(|x|+1) on 57 of 185 feature rows,
        3 GEMMs (185->1024 -> 1024->1024 -> 1024->1),
        2 RMS-style norms (mean already folded) + relu (+g,be affine).
  * Device layout: activations transposed (hidden on partitions, batch on
    free dim). LN stats (sum of squares over hidden) via ones-vector matmul
    on TensorE; rsqrt broadcast back via a rank-1 outer-product matmul.
  * All matmul operands and the elementwise chain are fp16: matmuls stream
    1 cyc/col with FWL; DVE elementwise ops hit the 2x_1p packed mode.
  * Scalar engine uses only {Ln, Exp, Identity} — one activation table
    set (natural_log_exp_and_others), so zero mid-kernel table swaps.
    rsqrt(v) is computed as exp(-0.5*ln(v)) on [1,CH] stats rows;
    sign(x)*ln(|x|+1) is ln(|x|+1) with x's sign bit OR'd in (DVE).
  * LN scale applied as relu(H)*pv on DVE (pv > 0 commutes with relu);
    the LN2 scale factors all the way out of the linear L3 contraction:
    y = rs2 * (w3^T relu(H2)) + b3 — no second broadcast matmul.
  * Batch 65536 is sharded 8192/core across 8 NeuronCores (pure DP).
"""
import numpy as np

import concourse.bass as bass
import concourse.tile as tile
from concourse import bacc, mybir
from concourse.bass_utils import run_bass_kernel_spmd

# The act-table insertion pass keeps a "current set" and greedily switches to
# the FIRST act_info.json set containing a missing function. With Ln and Exp
# both needed, that thrashes exp_and_others <-> natural_log (~2.7us per swap,
# 4 swaps per chunk). Every function this kernel uses (ln, exp, sign,
# identity, relu, copy, square) lives in natural_log_exp_and_others, so blank
# out every other set (preserving dict order = act_func_set_id indices) to
# pin the pass to the one set that covers the whole program.
_ONE_SET = "natural_log_exp_and_others"
_orig_get_tables = bacc.get_activation_tables


def _pinned_tables(arch):
    t = dict(_orig_get_tables(arch))
    if _ONE_SET not in t:  # unexpected act_info — keep stock behavior
        return t
    return {k: (v if k == _ONE_SET else set()) for k, v in t.items()}


bacc.get_activation_tables = _pinned_tables

AF = mybir.ActivationFunctionType
ALU = mybir.AluOpType
F32 = mybir.dt.float32
F16 = mybir.dt.float16
I16 = mybir.dt.int16

B = 65536
N_CORES = 8
B_CORE = B // N_CORES          # 8192
CH = 512                       # batch chunk (one PSUM bank wide)
NCH = B_CORE // CH             # 16
HID = 1024
MT = HID // 128                # 8 hidden m-tiles
KA, KC = 128, 57               # feature K tiles (125+3pad | 57 transformed)
EPS = 1e-5
LN2 = float(np.log(2.0))


# ---------------------------------------------------------------- host folds
def _fold_weights(inp):
    f8 = lambda x: np.asarray(x, np.float64)
    W1 = f8(inp["W1"]); b1 = f8(inp["b1"])
    emb_kc = f8(inp["emb_kc"]); emb_nl = f8(inp["emb_nl"])
    op_W = f8(inp["op_W"]); op_b = f8(inp["op_b"])
    emb_c = f8(inp["emb_contig"]); emb_s = f8(inp["emb_scalar"])
    emb_i = f8(inp["emb_indirect"])
    H = W1.shape[1]
    rows_A = []
    bias = b1.copy()
    rows_A.append(emb_kc @ W1[0:16])
    rows_A.append(emb_nl @ W1[16:32])
    W1_op = W1[32:944].reshape(57, 16, H)
    rows_A.append(np.einsum("ij,ijh->ih", op_W, W1_op))
    bias += np.einsum("ij,ijh->h", op_b, W1_op)
    rd_f2, rd_bool, rd_ss = [], [], []
    wd_f2, wd_bool, wd_ss = [], [], []
    for base, f2l, booll, ssl in ((947, rd_f2, rd_bool, rd_ss),
                                  (1027, wd_f2, wd_bool, wd_ss)):
        for d in range(4):
            Wd = W1[base + 20 * d: base + 20 * d + 20]
            f2l.append(Wd[0:2])
            ssl.append(Wd[2:8] / LN2)
            rows_b = []
            for e, sl in ((emb_c, slice(8, 12)), (emb_s, slice(12, 16)),
                          (emb_i, slice(16, 20))):
                rows_b.append((e[1] - e[0]) @ Wd[sl])
                bias += e[0] @ Wd[sl]
            booll.append(np.stack(rows_b))
    rows_A += [np.concatenate(rd_f2), np.concatenate(rd_bool),
               np.concatenate(wd_f2), np.concatenate(wd_bool),
               W1[1110:1112]]
    A = np.concatenate(rows_A)
    C = np.concatenate([W1[944:947] / LN2, W1[1107:1110] / LN2,
                        W1[1112:1115] / LN2,
                        np.concatenate(rd_ss), np.concatenate(wd_ss)])
    W1_eff = np.concatenate([A, np.zeros((3, H)), C])       # [185, H]
    W1c = W1_eff - W1_eff.mean(axis=1, keepdims=True)
    bc1 = bias - bias.mean()
    W2 = f8(inp["W2"]); b2 = f8(inp["b2"])
    W2c = W2 - W2.mean(axis=1, keepdims=True)
    bc2 = b2 - b2.mean()
    return (W1c.astype(np.float16), bc1.astype(np.float32),
            W2c.astype(np.float16), bc2.astype(np.float32))


def _build_xt(inp):
    Bn = inp["op_vec"].shape[0]
    kc = np.asarray(inp["kernel_category_idx"]).astype(np.int64)
    nl = np.asarray(inp["num_of_loops_idx"]).astype(np.int64)
    f = lambda k: np.asarray(inp[k], np.float32)
    XT = np.zeros((KA + KC, Bn), np.float16)
    XT[0:10] = (np.arange(10)[:, None] == kc[None, :])
    XT[10:26] = (np.arange(16)[:, None] == nl[None, :])
    XT[26:83] = f("op_vec").T
    XT[83:91] = f("read_dep_float")[:, :, 0:2].reshape(Bn, 8).T
    XT[91:103] = np.asarray(inp["read_dep_bools"]).reshape(Bn, 12).T
    XT[103:111] = f("write_dep_float")[:, :, 0:2].reshape(Bn, 8).T
    XT[111:123] = np.asarray(inp["write_dep_bools"]).reshape(Bn, 12).T
    XT[123:125] = f("rest_vec")[:, 3:5].T
    XT[128:131] = f("size_hints").T
    XT[131:137] = f("rest_vec")[:, [0, 1, 2, 5, 6, 7]].T
    XT[137:161] = f("read_dep_float")[:, :, 2:8].reshape(Bn, 24).T
    XT[161:185] = f("write_dep_float")[:, :, 2:8].reshape(Bn, 24).T
    return XT


def _pack128(v):
    """[1024] -> [128, 8] with v[m*128+p] at [p, m]."""
    return np.ascontiguousarray(np.asarray(v, np.float32).reshape(8, 128).T)


# ---------------------------------------------------------------- device prog
DEFAULT_CFG = dict(h_bufs=3, sq_bufs=4, r1_bufs=2, r2_bufs=2,
                   ps_mm_bufs=4, ps_st_bufs=2, ps_vec_bufs=2,
                   xin_bufs=3, xr_bufs=3, small_bufs=2, l2_pairs=2)


def build_program(simple_affine, loop_iters=None, cfg=None):
    """Build the per-core bass program, software-pipelined 3 chunks deep.

    Pipeline stages per chunk c (emitted stage-shifted so the PE stream of
    chunk c's L2 GEMMs overlaps chunk c+1's L1 and chunk c-1's L3):
      S0(c): DMA x + transform u = sign(x)*ln(|x|+1)
      S1(c): L1 matmuls + bias copy-out + per-m square + LN1 stats + rsqrt1
      S2(c): LN1 broadcast + scale+relu -> R1
      S3(c): L2 matmuls (m-pairs over k) + copy-out + square + stats + rsqrt2
      S4(c): LN2 broadcast + scale+relu -> R2
      S5(c): L3 matmuls + bias + DMA out
    """
    cfg = {**DEFAULT_CFG, **(cfg or {})}
    nc = bacc.Bacc("TRN2", target_bir_lowering=False, debug=False)
    xt = nc.dram_tensor("xt", [KA + KC, B_CORE], F16, kind="ExternalInput")
    w1 = nc.dram_tensor("w1", [KA + KC, HID], F16, kind="ExternalInput")
    w2 = nc.dram_tensor("w2", [HID, HID], F16, kind="ExternalInput")
    w3p = nc.dram_tensor("w3p", [128, MT], F16, kind="ExternalInput")
    bc1p = nc.dram_tensor("bc1p", [128, MT], F32, kind="ExternalInput")
    bc2p = nc.dram_tensor("bc2p", [128, MT], F32, kind="ExternalInput")
    g1p = nc.dram_tensor("g1p", [128, MT], F32, kind="ExternalInput")
    be1p = nc.dram_tensor("be1p", [128, MT], F32, kind="ExternalInput")
    g2p = nc.dram_tensor("g2p", [128, MT], F32, kind="ExternalInput")
    be2p = nc.dram_tensor("be2p", [128, MT], F32, kind="ExternalInput")
    b3t = nc.dram_tensor("b3t", [1, 1], F32, kind="ExternalInput")
    y = nc.dram_tensor("y", [1, B_CORE], F32, kind="ExternalOutput")

    from contextlib import ExitStack
    with tile.TileContext(nc) as tc, ExitStack() as ctx, \
            nc.allow_low_precision(reason="fp16 rounding is intentional"):
        const = ctx.enter_context(tc.tile_pool(name="const", bufs=1))
        xin = ctx.enter_context(tc.tile_pool(name="xin", bufs=cfg["xin_bufs"]))
        xr = ctx.enter_context(tc.tile_pool(name="xr", bufs=cfg["xr_bufs"]))
        bigH = ctx.enter_context(tc.tile_pool(name="bigH", bufs=cfg["h_bufs"]))
        sqp = ctx.enter_context(tc.tile_pool(name="sqp", bufs=cfg["sq_bufs"]))
        bigR1 = ctx.enter_context(tc.tile_pool(name="bigR1", bufs=cfg["r1_bufs"]))
        bigR2 = ctx.enter_context(tc.tile_pool(name="bigR2", bufs=cfg["r2_bufs"]))
        small = ctx.enter_context(tc.tile_pool(name="small", bufs=cfg["small_bufs"]))
        ps_mm = ctx.enter_context(tc.tile_pool(name="ps_mm", bufs=cfg["ps_mm_bufs"], space="PSUM"))
        ps_st = ctx.enter_context(tc.tile_pool(name="ps_st", bufs=cfg["ps_st_bufs"], space="PSUM"))
        ps_vec = ctx.enter_context(tc.tile_pool(name="ps_vec", bufs=cfg["ps_vec_bufs"], space="PSUM"))

        # ---- one-time setup: everything arrives in fp16, no staging copies
        # (w1 + the first chunk's x are queued before the 2MB of w2 tiles so
        # the first L1 matmul isn't stuck behind them on the DMA queue)
        w1a_r = const.tile([128, HID], F16, tag="w1a")
        nc.sync.dma_start(w1a_r[:], w1.ap()[0:128, :])
        w1c_r = const.tile([KC, HID], F16, tag="w1c")
        nc.sync.dma_start(w1c_r[:], w1.ap()[128:185, :])
        w2r = []
        for k in range(MT):
            t = const.tile([128, HID], F16, tag=f"w2r{k}")
            nc.sync.dma_start(t[:], w2.ap()[k * 128:(k + 1) * 128, :])
            w2r.append(t)
        w3p_r = const.tile([128, MT], F16, tag="w3p")
        nc.sync.dma_start(w3p_r[:], w3p.ap())
        w3s32 = const.tile([128, MT], F32, tag="w3s32")
        nc.vector.tensor_copy(w3s32[:], w3p_r[:])

        def load_small(name, dram):
            t = const.tile([128, MT], F32, tag=name)
            nc.sync.dma_start(t[:], dram.ap())
            return t
        bc1s = load_small("bc1s", bc1p); bc2s = load_small("bc2s", bc2p)
        g1s = load_small("g1s", g1p); be1s = load_small("be1s", be1p)
        g2s = load_small("g2s", g2p); be2s = load_small("be2s", be2p)
        b3s = const.tile([1, 1], F32, tag="b3s")
        nc.sync.dma_start(b3s[:], b3t.ap())
        ones_st = const.tile([128, 1], F32, tag="ones_st")
        nc.vector.memset(ones_st[:], 1.0)
        ones_col = const.tile([128, 1], F16, tag="ones_col")
        nc.vector.tensor_copy(ones_col[:], ones_st[:])
        ones_rst = const.tile([1, 128], F32, tag="ones_rst")
        nc.vector.memset(ones_rst[:], 1.0)
        ones_row = const.tile([1, 128], F16, tag="ones_row")
        nc.vector.tensor_copy(ones_row[:], ones_rst[:])
        eps_t = const.tile([1, 1], F32, tag="eps_t")
        nc.vector.memset(eps_t[:], EPS)

        def rsqrt_stats(pst, tag, dt=F16):
            """pst [1, CH] f32 = sum(H^2) over hidden.
            Returns [1, CH] = 1/sqrt(pst/HID + EPS) via exp(-0.5*ln(v))
            (keeps scalar engine inside one activation table set)."""
            tln = small.tile([1, CH], F32, tag=f"tln{tag}")
            nc.scalar.activation(tln[:], pst[:], AF.Ln,
                                 bias=eps_t[:], scale=1.0 / HID)
            rs = small.tile([1, CH], dt, tag=f"rs{tag}")
            nc.scalar.activation(rs[:], tln[:], AF.Exp, scale=-0.5)
            return rs

        def scale_relu(Hb, rs, g_s, be_s, out_pool, out_tag, tag):
            """R = relu(H * bcast(rs)) per m-tile. Returns f16 tile."""
            pv = ps_vec.tile([128, CH], F32, tag="pv")
            nc.tensor.matmul(pv[:], ones_row[:], rs[:], start=True, stop=True)
            pvs = small.tile([128, CH], F16, tag=f"pvs{tag}")
            nc.vector.tensor_copy(pvs[:], pv[:])
            Rb = out_pool.tile([128, MT * CH], F16, tag=out_tag)
            for m in range(MT):
                sl = slice(m * CH, (m + 1) * CH)
                nc.vector.tensor_mul(Hb[:, sl], Hb[:, sl], pvs[:])
                if simple_affine:
                    nc.vector.tensor_scalar(
                        out=Rb[:, sl], in0=Hb[:, sl],
                        scalar1=0.0, scalar2=None, op0=ALU.max)
                else:
                    nc.scalar.activation(Rb[:, sl], Hb[:, sl], AF.Relu,
                                         bias=be_s[:, m:m + 1],
                                         scale=g_s[:, m:m + 1])
            return Rb

        def make_state():
            return {}

        def s0_xprep(st, c):
            x1 = xin.tile([128, CH], F16, tag="x1")
            nc.sync.dma_start(x1[:], xt.ap()[0:128, c * CH:(c + 1) * CH])
            x2 = xin.tile([KC, CH], F16, tag="x2")
            nc.sync.dma_start(x2[:], xt.ap()[128:185, c * CH:(c + 1) * CH])
            # u = sign(x)*ln(|x|+1) on the 57 transformed rows
            xab = xr.tile([KC, CH], F16, tag="xab")
            nc.vector.tensor_scalar(
                out=xab[:].bitcast(I16),
                in0=x2[:].bitcast(I16),
                scalar1=0x7FFF, scalar2=None, op0=ALU.bitwise_and)
            xln = xr.tile([KC, CH], F16, tag="xln")
            nc.scalar.activation(xln[:], xab[:], AF.Ln, bias=1.0)
            # sign(x)*ln(|x|+1) == ln(|x|+1) with x's sign bit OR'd in
            # (ln(|x|+1) >= 0 always), so no scalar Sign / DVE mul needed
            xsb = xr.tile([KC, CH], F16, tag="xsb")
            nc.vector.tensor_scalar(
                out=xsb[:].bitcast(I16),
                in0=x2[:].bitcast(I16),
                scalar1=-0x8000, scalar2=None, op0=ALU.bitwise_and)
            x2r = xr.tile([KC, CH], F16, tag="x2r")
            nc.vector.tensor_tensor(
                out=x2r[:].bitcast(I16), in0=xln[:].bitcast(I16),
                in1=xsb[:].bitcast(I16), op=ALU.bitwise_or)
            st[("x", c)] = (x1, x2r)

        def acc_tree_add(accA, accB, t, m, tag):
            """Accumulate per-m tiles into accA (m 0-3, DVE) and accB
            (m 4-7, GPSIMD) as two parallel chains; caller joins at m==7
            with a final DVE add. t=None means the value was already
            written directly into the chain head by the producer."""
            if t is not None:
                if m < 4:
                    nc.vector.tensor_add(accA[:], accA[:], t[:])
                else:
                    nc.gpsimd.tensor_add(accB[:], accB[:], t[:])
            if m == MT - 1:
                nc.vector.tensor_add(accA[:], accA[:], accB[:])

        def s1_l1(st, c):
            x1, x2r = st.pop(("x", c))
            H1 = bigH.tile([128, MT * CH], F16, tag="H1")
            pst = ps_st.tile([1, CH], F32, tag="pst")
            for m in range(MT):
                p1 = ps_mm.tile([128, CH], F32, tag="pmm")
                nc.tensor.matmul(p1[:], w1a_r[:, m * 128:(m + 1) * 128],
                                 x1[:], start=True, stop=False)
                nc.tensor.matmul(p1[:], w1c_r[:, m * 128:(m + 1) * 128],
                                 x2r[:], start=False, stop=True)
                sl = slice(m * CH, (m + 1) * CH)
                nc.scalar.activation(H1[:, sl], p1[:],
                                     AF.Identity, bias=bc1s[:, m:m + 1])
                sq = sqp.tile([128, CH], F16, tag="sqa")
                nc.vector.tensor_mul(sq[:], H1[:, sl], H1[:, sl])
                nc.tensor.matmul(pst[:], ones_col[:], sq[:],
                                 start=(m == 0), stop=(m == MT - 1))
            st[("h1", c)] = H1
            st[("rs1", c)] = rsqrt_stats(pst, "a")

        def s2_ln1(st, c):
            H1 = st.pop(("h1", c))
            rs = st.pop(("rs1", c))
            st[("r1", c)] = scale_relu(H1, rs, g1s, be1s, bigR1, "R1", "a")

        def s3_l2(st, c):
            """L2 + LN2 stats. For the simple-affine case, the LN2 rescale
            is per-column, so it commutes past relu AND the (linear) L3
            contraction: y = rs2 * (w3.T relu(H2)) + b3. Q2 = relu(H2) is
            produced here per m-tile; no broadcast matmul / pv2 needed."""
            R1 = st.pop(("r1", c))
            H2 = bigH.tile([128, MT * CH], F16, tag="H2")
            pst = ps_st.tile([1, CH], F32, tag="pst")
            Q2 = bigR2.tile([128, MT * CH], F16, tag="R2")
            npair = cfg["l2_pairs"]
            for mp in range(0, MT, npair):
                ms = range(mp, mp + npair)
                p2 = {}
                for m in ms:
                    p2m = ps_mm.tile([128, CH], F32, tag="pmm")
                    p2[m] = p2m
                for k in range(MT):
                    for m in ms:
                        nc.tensor.matmul(
                            p2[m][:], w2r[k][:, m * 128:(m + 1) * 128],
                            R1[:, k * CH:(k + 1) * CH],
                            start=(k == 0), stop=(k == MT - 1))
                for m in ms:
                    sl = slice(m * CH, (m + 1) * CH)
                    nc.scalar.activation(H2[:, sl], p2[m][:],
                                         AF.Identity, bias=bc2s[:, m:m + 1])
                    sq = sqp.tile([128, CH], F16, tag="sqb")
                    nc.vector.tensor_mul(sq[:], H2[:, sl], H2[:, sl])
                    nc.tensor.matmul(pst[:], ones_col[:], sq[:],
                                     start=(m == 0), stop=(m == MT - 1))
                    if simple_affine:
                        nc.vector.tensor_scalar(
                            out=Q2[:, sl], in0=H2[:, sl],
                            scalar1=0.0, scalar2=None, op0=ALU.max)
            st[("h2", c)] = H2
            st[("q2", c)] = Q2
            st[("rs2", c)] = rsqrt_stats(pst, "b", dt=F32)

        def s5_l3(st, c):
            H2 = st.pop(("h2", c))
            rs2 = st.pop(("rs2", c))
            Q2 = st.pop(("q2", c))
            p3 = ps_st.tile([1, CH], F32, tag="pst")
            if simple_affine:
                for k in range(MT):
                    nc.tensor.matmul(p3[:], w3p_r[:, k:k + 1],
                                     Q2[:, k * CH:(k + 1) * CH],
                                     start=(k == 0), stop=(k == MT - 1))
            else:
                rs16 = small.tile([1, CH], F16, tag="rs2f16")
                nc.vector.tensor_copy(rs16[:], rs2[:])
                R2 = scale_relu(H2, rs16, g2s, be2s, bigR2, "R2x", "b")
                for k in range(MT):
                    nc.tensor.matmul(p3[:], w3p_r[:, k:k + 1],
                                     R2[:, k * CH:(k + 1) * CH],
                                     start=(k == 0), stop=(k == MT - 1))
            osb = small.tile([1, CH], F32, tag="osb")
            if simple_affine:
                yv = small.tile([1, CH], F32, tag="yv")
                nc.vector.tensor_mul(yv[:], p3[:], rs2[:])
                nc.scalar.activation(osb[:], yv[:], AF.Identity, bias=b3s[:])
            else:
                nc.scalar.activation(osb[:], p3[:], AF.Identity, bias=b3s[:])
            nc.sync.dma_start(y.ap()[0:1, c * CH:(c + 1) * CH], osb[:])

        def pipeline(st=None):
            if st is None:
                st = make_state()
                s0_xprep(st, 0)
            for i in range(NCH + 1):
                if i + 1 < NCH:
                    s0_xprep(st, i + 1)
                if i < NCH:
                    s1_l1(st, i)
                if 0 <= i - 1 < NCH:
                    s2_ln1(st, i - 1)
                    s3_l2(st, i - 1)
                    s5_l3(st, i - 1)

        if loop_iters is None:
            st0 = make_state()
            with tc.high_priority():
                s0_xprep(st0, 0)
            pipeline(st0)
        else:
            hint = tuple(
                mybir.EngineType(e) for e in
                ("PE", "Activation", "DVE", "SP", "Pool")
            ) if cfg.get("loop_hints") else ()
            with tc.For_i(0, loop_iters, 1,
                          staggered_reset=cfg.get("loop_stagger", False),
                          hint_engines=hint):
                pipeline()
    nc.compile()
    return nc


# ---------------------------------------------------------------- entry point
_CACHE = {}


BEST_CFG = dict()


def _get_program(simple_affine):
    key = ("prog", simple_affine)
    if key not in _CACHE:
        _CACHE[key] = build_program(simple_affine, cfg=BEST_CFG)
    return _CACHE[key]


def make_in_maps(inputs):
    inp = {k: np.asarray(v) for k, v in inputs.items()}
    W1c, bc1, W2c, bc2 = _fold_weights(inp)
    XT = _build_xt(inp)
    g1 = np.asarray(inp["g1"], np.float32); be1 = np.asarray(inp["be1"], np.float32)
    g2 = np.asarray(inp["g2"], np.float32); be2 = np.asarray(inp["be2"], np.float32)
    simple_affine = bool(
        np.all(g1 == 1.0) and np.all(g2 == 1.0)
        and np.all(be1 == 0.0) and np.all(be2 == 0.0))
    W3 = np.asarray(inp["W3"], np.float32)
    b3 = np.asarray(inp["b3"], np.float32)
    shared = {
        "w1": W1c, "w2": W2c,
        "w3p": _pack128(W3[:, 0]).astype(np.float16),
        "bc1p": _pack128(bc1), "bc2p": _pack128(bc2),
        "g1p": _pack128(g1), "be1p": _pack128(be1),
        "g2p": _pack128(g2), "be2p": _pack128(be2),
        "b3t": b3.reshape(1, 1),
    }
    in_maps = []
    for c in range(N_CORES):
        m = dict(shared)
        m["xt"] = np.ascontiguousarray(XT[:, c * B_CORE:(c + 1) * B_CORE])
        in_maps.append(m)
    return in_maps, simple_affine


def kernel(**inputs) -> np.ndarray:
    in_maps, simple_affine = make_in_maps(inputs)
    nc = _get_program(simple_affine)
    res = run_bass_kernel_spmd(nc, in_maps, core_ids=list(range(N_CORES)))
    y = np.concatenate([r["y"][0] for r in res.results])
    return y.reshape(B, 1).astype(np.float32)


if __name__ == "__main__":
    import jax
    import reference
    cpu = jax.devices("cpu")[0]
    with jax.default_device(cpu):
        inp = reference.setup_inputs()
        ref = np.asarray(reference.reference(**inp))
    out = kernel(**{k: np.asarray(v) for k, v in inp.items()})
    err = np.abs(out - ref)
    scale = np.abs(ref).max()
    print("max_abs", err.max(), "rel(vs scale)", err.max() / scale,
          "mean_rel", (err / (np.abs(ref) + 1e-6)).mean())


# revision 29
# speedup vs baseline: 1.2307x; 1.1925x over previous
"""Trainium2 Bass kernel for nn_Autotuner_FFN (dense MLP, 8-core data parallel).

Strategy:
  * Host folds all embedding tables / 57 op-linears / log2-scalings and the
    LayerNorm mean-centerings into one effective first-layer weight matrix
    W1_eff [185, 1024] (+ centered biases). One-hot index encodings become
    extra GEMM rows. Per-element device work shrinks to:
        u = sign(x)*ln(|x|+1) on 57 of 185 feature rows,
        3 GEMMs (185->1024 -> 1024->1024 -> 1024->1),
        2 RMS-style norms (mean already folded) + relu (+g,be affine).
  * Device layout: activations transposed (hidden on partitions, batch on
    free dim). LN stats (sum of squares over hidden) via ones-vector matmul
    on TensorE; rsqrt broadcast back via a rank-1 outer-product matmul.
  * All matmul operands and the elementwise chain are fp16: matmuls stream
    1 cyc/col with FWL; DVE elementwise ops hit the 2x_1p packed mode.
  * Scalar engine uses only {Ln, Exp, Identity} — one activation table
    set (natural_log_exp_and_others), so zero mid-kernel table swaps.
    rsqrt(v) is computed as exp(-0.5*ln(v)) on [1,CH] stats rows;
    sign(x)*ln(|x|+1) is ln(|x|+1) with x's sign bit OR'd in (DVE).
  * LN scale applied as relu(H)*pv on DVE (pv > 0 commutes with relu);
    the LN2 scale factors all the way out of the linear L3 contraction:
    y = rs2 * (w3^T relu(H2)) + b3 — no second broadcast matmul.
  * Batch 65536 is sharded 8192/core across 8 NeuronCores (pure DP).
"""
import numpy as np

import concourse.bass as bass
import concourse.tile as tile
from concourse import bacc, mybir
from concourse.bass_utils import run_bass_kernel_spmd

# The act-table insertion pass keeps a "current set" and greedily switches to
# the FIRST act_info.json set containing a missing function. With Ln and Exp
# both needed, that thrashes exp_and_others <-> natural_log (~2.7us per swap,
# 4 swaps per chunk). Every function this kernel uses (ln, exp, sign,
# identity, relu, copy, square) lives in natural_log_exp_and_others, so blank
# out every other set (preserving dict order = act_func_set_id indices) to
# pin the pass to the one set that covers the whole program.
_ONE_SET = "natural_log_exp_and_others"
_orig_get_tables = bacc.get_activation_tables


def _pinned_tables(arch):
    t = dict(_orig_get_tables(arch))
    if _ONE_SET not in t:  # unexpected act_info — keep stock behavior
        return t
    return {k: (v if k == _ONE_SET else set()) for k, v in t.items()}


bacc.get_activation_tables = _pinned_tables

AF = mybir.ActivationFunctionType
ALU = mybir.AluOpType
F32 = mybir.dt.float32
F16 = mybir.dt.float16
I16 = mybir.dt.int16

B = 65536
N_CORES = 8
B_CORE = B // N_CORES          # 8192
CH = 512                       # batch chunk (one PSUM bank wide)
NCH = B_CORE // CH             # 16
HID = 1024
MT = HID // 128                # 8 hidden m-tiles
KA, KC = 128, 57               # feature K tiles (125+3pad | 57 transformed)
EPS = 1e-5
LN2 = float(np.log(2.0))


# ---------------------------------------------------------------- host folds
def _fold_weights(inp):
    f8 = lambda x: np.asarray(x, np.float64)
    W1 = f8(inp["W1"]); b1 = f8(inp["b1"])
    emb_kc = f8(inp["emb_kc"]); emb_nl = f8(inp["emb_nl"])
    op_W = f8(inp["op_W"]); op_b = f8(inp["op_b"])
    emb_c = f8(inp["emb_contig"]); emb_s = f8(inp["emb_scalar"])
    emb_i = f8(inp["emb_indirect"])
    H = W1.shape[1]
    rows_A = []
    bias = b1.copy()
    rows_A.append(emb_kc @ W1[0:16])
    rows_A.append(emb_nl @ W1[16:32])
    W1_op = W1[32:944].reshape(57, 16, H)
    rows_A.append(np.einsum("ij,ijh->ih", op_W, W1_op))
    bias += np.einsum("ij,ijh->h", op_b, W1_op)
    rd_f2, rd_bool, rd_ss = [], [], []
    wd_f2, wd_bool, wd_ss = [], [], []
    for base, f2l, booll, ssl in ((947, rd_f2, rd_bool, rd_ss),
                                  (1027, wd_f2, wd_bool, wd_ss)):
        for d in range(4):
            Wd = W1[base + 20 * d: base + 20 * d + 20]
            f2l.append(Wd[0:2])
            ssl.append(Wd[2:8] / LN2)
            rows_b = []
            for e, sl in ((emb_c, slice(8, 12)), (emb_s, slice(12, 16)),
                          (emb_i, slice(16, 20))):
                rows_b.append((e[1] - e[0]) @ Wd[sl])
                bias += e[0] @ Wd[sl]
            booll.append(np.stack(rows_b))
    rows_A += [np.concatenate(rd_f2), np.concatenate(rd_bool),
               np.concatenate(wd_f2), np.concatenate(wd_bool),
               W1[1110:1112]]
    A = np.concatenate(rows_A)
    C = np.concatenate([W1[944:947] / LN2, W1[1107:1110] / LN2,
                        W1[1112:1115] / LN2,
                        np.concatenate(rd_ss), np.concatenate(wd_ss)])
    W1_eff = np.concatenate([A, np.zeros((3, H)), C])       # [185, H]
    W1c = W1_eff - W1_eff.mean(axis=1, keepdims=True)
    bc1 = bias - bias.mean()
    W2 = f8(inp["W2"]); b2 = f8(inp["b2"])
    W2c = W2 - W2.mean(axis=1, keepdims=True)
    bc2 = b2 - b2.mean()
    return (W1c.astype(np.float16), bc1.astype(np.float32),
            W2c.astype(np.float16), bc2.astype(np.float32))


def _build_xt(inp):
    Bn = inp["op_vec"].shape[0]
    kc = np.asarray(inp["kernel_category_idx"]).astype(np.int64)
    nl = np.asarray(inp["num_of_loops_idx"]).astype(np.int64)
    f = lambda k: np.asarray(inp[k], np.float32)
    XT = np.zeros((KA + KC, Bn), np.float16)
    XT[0:10] = (np.arange(10)[:, None] == kc[None, :])
    XT[10:26] = (np.arange(16)[:, None] == nl[None, :])
    XT[26:83] = f("op_vec").T
    XT[83:91] = f("read_dep_float")[:, :, 0:2].reshape(Bn, 8).T
    XT[91:103] = np.asarray(inp["read_dep_bools"]).reshape(Bn, 12).T
    XT[103:111] = f("write_dep_float")[:, :, 0:2].reshape(Bn, 8).T
    XT[111:123] = np.asarray(inp["write_dep_bools"]).reshape(Bn, 12).T
    XT[123:125] = f("rest_vec")[:, 3:5].T
    XT[128:131] = f("size_hints").T
    XT[131:137] = f("rest_vec")[:, [0, 1, 2, 5, 6, 7]].T
    XT[137:161] = f("read_dep_float")[:, :, 2:8].reshape(Bn, 24).T
    XT[161:185] = f("write_dep_float")[:, :, 2:8].reshape(Bn, 24).T
    return XT


def _pack128(v):
    """[1024] -> [128, 8] with v[m*128+p] at [p, m]."""
    return np.ascontiguousarray(np.asarray(v, np.float32).reshape(8, 128).T)


# ---------------------------------------------------------------- device prog
DEFAULT_CFG = dict(h_bufs=3, sq_bufs=6, r1_bufs=2, r2_bufs=2,
                   ps_mm_bufs=4, ps_st_bufs=2, ps_vec_bufs=2,
                   xin_bufs=3, xr_bufs=3, small_bufs=2, l2_pairs=2)


def build_program(simple_affine, loop_iters=None, cfg=None):
    """Build the per-core bass program, software-pipelined 3 chunks deep.

    Pipeline stages per chunk c (emitted stage-shifted so the PE stream of
    chunk c's L2 GEMMs overlaps chunk c+1's L1 and chunk c-1's L3):
      S0(c): DMA x + transform u = sign(x)*ln(|x|+1)
      S1(c): L1 matmuls + bias copy-out + per-m square + LN1 stats + rsqrt1
      S2(c): LN1 broadcast + scale+relu -> R1
      S3(c): L2 matmuls (m-pairs over k) + copy-out + square + stats + rsqrt2
      S4(c): LN2 broadcast + scale+relu -> R2
      S5(c): L3 matmuls + bias + DMA out
    """
    cfg = {**DEFAULT_CFG, **(cfg or {})}
    nc = bacc.Bacc("TRN2", target_bir_lowering=False, debug=False)
    xt = nc.dram_tensor("xt", [KA + KC, B_CORE], F16, kind="ExternalInput")
    w1 = nc.dram_tensor("w1", [KA + KC, HID], F16, kind="ExternalInput")
    w2 = nc.dram_tensor("w2", [HID, HID], F16, kind="ExternalInput")
    w3p = nc.dram_tensor("w3p", [128, MT], F16, kind="ExternalInput")
    bc1p = nc.dram_tensor("bc1p", [128, MT], F32, kind="ExternalInput")
    bc2p = nc.dram_tensor("bc2p", [128, MT], F32, kind="ExternalInput")
    g1p = nc.dram_tensor("g1p", [128, MT], F32, kind="ExternalInput")
    be1p = nc.dram_tensor("be1p", [128, MT], F32, kind="ExternalInput")
    g2p = nc.dram_tensor("g2p", [128, MT], F32, kind="ExternalInput")
    be2p = nc.dram_tensor("be2p", [128, MT], F32, kind="ExternalInput")
    b3t = nc.dram_tensor("b3t", [1, 1], F32, kind="ExternalInput")
    y = nc.dram_tensor("y", [1, B_CORE], F32, kind="ExternalOutput")

    from contextlib import ExitStack
    with tile.TileContext(nc) as tc, ExitStack() as ctx, \
            nc.allow_low_precision(reason="fp16 rounding is intentional"):
        const = ctx.enter_context(tc.tile_pool(name="const", bufs=1))
        xin = ctx.enter_context(tc.tile_pool(name="xin", bufs=cfg["xin_bufs"]))
        xr = ctx.enter_context(tc.tile_pool(name="xr", bufs=cfg["xr_bufs"]))
        bigH = ctx.enter_context(tc.tile_pool(name="bigH", bufs=cfg["h_bufs"]))
        sqp = ctx.enter_context(tc.tile_pool(name="sqp", bufs=cfg["sq_bufs"]))
        bigR1 = ctx.enter_context(tc.tile_pool(name="bigR1", bufs=cfg["r1_bufs"]))
        bigR2 = ctx.enter_context(tc.tile_pool(name="bigR2", bufs=cfg["r2_bufs"]))
        small = ctx.enter_context(tc.tile_pool(name="small", bufs=cfg["small_bufs"]))
        ps_mm = ctx.enter_context(tc.tile_pool(name="ps_mm", bufs=cfg["ps_mm_bufs"], space="PSUM"))
        ps_st = ctx.enter_context(tc.tile_pool(name="ps_st", bufs=cfg["ps_st_bufs"], space="PSUM"))
        ps_vec = ctx.enter_context(tc.tile_pool(name="ps_vec", bufs=cfg["ps_vec_bufs"], space="PSUM"))

        # ---- one-time setup: everything arrives in fp16, no staging copies
        # (w1 + the first chunk's x are queued before the 2MB of w2 tiles so
        # the first L1 matmul isn't stuck behind them on the DMA queue)
        w1a_r = const.tile([128, HID], F16, tag="w1a")
        nc.sync.dma_start(w1a_r[:], w1.ap()[0:128, :])
        w1c_r = const.tile([KC, HID], F16, tag="w1c")
        nc.sync.dma_start(w1c_r[:], w1.ap()[128:185, :])
        w2r = []
        for k in range(MT):
            t = const.tile([128, HID], F16, tag=f"w2r{k}")
            nc.sync.dma_start(t[:], w2.ap()[k * 128:(k + 1) * 128, :])
            w2r.append(t)
        w3p_r = const.tile([128, MT], F16, tag="w3p")
        nc.sync.dma_start(w3p_r[:], w3p.ap())
        w3s32 = const.tile([128, MT], F32, tag="w3s32")
        nc.vector.tensor_copy(w3s32[:], w3p_r[:])

        def load_small(name, dram):
            t = const.tile([128, MT], F32, tag=name)
            nc.sync.dma_start(t[:], dram.ap())
            return t
        bc1s = load_small("bc1s", bc1p); bc2s = load_small("bc2s", bc2p)
        g1s = load_small("g1s", g1p); be1s = load_small("be1s", be1p)
        g2s = load_small("g2s", g2p); be2s = load_small("be2s", be2p)
        b3s = const.tile([1, 1], F32, tag="b3s")
        nc.sync.dma_start(b3s[:], b3t.ap())
        ones_st = const.tile([128, 1], F32, tag="ones_st")
        nc.vector.memset(ones_st[:], 1.0)
        ones_col = const.tile([128, 1], F16, tag="ones_col")
        nc.vector.tensor_copy(ones_col[:], ones_st[:])
        ones_rst = const.tile([1, 128], F32, tag="ones_rst")
        nc.vector.memset(ones_rst[:], 1.0)
        ones_row = const.tile([1, 128], F16, tag="ones_row")
        nc.vector.tensor_copy(ones_row[:], ones_rst[:])
        eps_t = const.tile([1, 1], F32, tag="eps_t")
        nc.vector.memset(eps_t[:], EPS)

        def rsqrt_stats(pst, tag, dt=F16):
            """pst [1, CH] f32 = sum(H^2) over hidden.
            Returns [1, CH] = 1/sqrt(pst/HID + EPS) via exp(-0.5*ln(v))
            (keeps scalar engine inside one activation table set)."""
            tln = small.tile([1, CH], F32, tag=f"tln{tag}")
            nc.scalar.activation(tln[:], pst[:], AF.Ln,
                                 bias=eps_t[:], scale=1.0 / HID)
            rs = small.tile([1, CH], dt, tag=f"rs{tag}")
            nc.scalar.activation(rs[:], tln[:], AF.Exp, scale=-0.5)
            return rs

        def scale_relu(Hb, rs, g_s, be_s, out_pool, out_tag, tag):
            """R = relu(H * bcast(rs)) per m-tile. Returns f16 tile."""
            pv = ps_vec.tile([128, CH], F32, tag="pv")
            nc.tensor.matmul(pv[:], ones_row[:], rs[:], start=True, stop=True)
            pvs = small.tile([128, CH], F16, tag=f"pvs{tag}")
            nc.vector.tensor_copy(pvs[:], pv[:])
            Rb = out_pool.tile([128, MT * CH], F16, tag=out_tag)
            for m in range(MT):
                sl = slice(m * CH, (m + 1) * CH)
                nc.vector.tensor_mul(Hb[:, sl], Hb[:, sl], pvs[:])
                if simple_affine:
                    nc.vector.tensor_scalar(
                        out=Rb[:, sl], in0=Hb[:, sl],
                        scalar1=0.0, scalar2=None, op0=ALU.max)
                else:
                    nc.scalar.activation(Rb[:, sl], Hb[:, sl], AF.Relu,
                                         bias=be_s[:, m:m + 1],
                                         scale=g_s[:, m:m + 1])
            return Rb

        def make_state():
            return {}

        def s0_xprep(st, c):
            x1 = xin.tile([128, CH], F16, tag="x1")
            nc.sync.dma_start(x1[:], xt.ap()[0:128, c * CH:(c + 1) * CH])
            x2 = xin.tile([KC, CH], F16, tag="x2")
            nc.sync.dma_start(x2[:], xt.ap()[128:185, c * CH:(c + 1) * CH])
            # u = sign(x)*ln(|x|+1) on the 57 transformed rows
            xab = xr.tile([KC, CH], F16, tag="xab")
            nc.vector.tensor_scalar(
                out=xab[:].bitcast(I16),
                in0=x2[:].bitcast(I16),
                scalar1=0x7FFF, scalar2=None, op0=ALU.bitwise_and)
            xln = xr.tile([KC, CH], F16, tag="xln")
            nc.scalar.activation(xln[:], xab[:], AF.Ln, bias=1.0)
            # sign(x)*ln(|x|+1) == ln(|x|+1) with x's sign bit OR'd in
            # (ln(|x|+1) >= 0 always), so no scalar Sign / DVE mul needed
            xsb = xr.tile([KC, CH], F16, tag="xsb")
            nc.vector.tensor_scalar(
                out=xsb[:].bitcast(I16),
                in0=x2[:].bitcast(I16),
                scalar1=-0x8000, scalar2=None, op0=ALU.bitwise_and)
            x2r = xr.tile([KC, CH], F16, tag="x2r")
            nc.vector.tensor_tensor(
                out=x2r[:].bitcast(I16), in0=xln[:].bitcast(I16),
                in1=xsb[:].bitcast(I16), op=ALU.bitwise_or)
            st[("x", c)] = (x1, x2r)

        def acc_tree_add(accA, accB, t, m, tag):
            """Accumulate per-m tiles into accA (m 0-3, DVE) and accB
            (m 4-7, GPSIMD) as two parallel chains; caller joins at m==7
            with a final DVE add. t=None means the value was already
            written directly into the chain head by the producer."""
            if t is not None:
                if m < 4:
                    nc.vector.tensor_add(accA[:], accA[:], t[:])
                else:
                    nc.gpsimd.tensor_add(accB[:], accB[:], t[:])
            if m == MT - 1:
                nc.vector.tensor_add(accA[:], accA[:], accB[:])

        def s1_l1(st, c):
            x1, x2r = st.pop(("x", c))
            H1 = bigH.tile([128, MT * CH], F16, tag="H1")
            pst = ps_st.tile([1, CH], F32, tag="pst")
            for m in range(MT):
                p1 = ps_mm.tile([128, CH], F32, tag="pmm")
                nc.tensor.matmul(p1[:], w1a_r[:, m * 128:(m + 1) * 128],
                                 x1[:], start=True, stop=False)
                nc.tensor.matmul(p1[:], w1c_r[:, m * 128:(m + 1) * 128],
                                 x2r[:], start=False, stop=True)
                sl = slice(m * CH, (m + 1) * CH)
                nc.scalar.activation(H1[:, sl], p1[:],
                                     AF.Identity, bias=bc1s[:, m:m + 1])
                sq = sqp.tile([128, CH], F16, tag="sqa")
                nc.vector.tensor_mul(sq[:], H1[:, sl], H1[:, sl])
                # pair-sum adjacent m squares on DVE: halves the PE
                # stats stream (one independent add per pair, no chain)
                if m % 2 == 0:
                    sq_hold = sq
                else:
                    nc.vector.tensor_add(sq_hold[:], sq_hold[:], sq[:])
                    if (m // 2) % 2 == 0:
                        quad_hold = sq_hold
                    else:
                        nc.vector.tensor_add(quad_hold[:], quad_hold[:],
                                             sq_hold[:])
                        nc.tensor.matmul(pst[:], ones_col[:], quad_hold[:],
                                         start=(m == 3), stop=(m == MT - 1))
            st[("h1", c)] = H1
            st[("rs1", c)] = rsqrt_stats(pst, "a")

        def s2_ln1(st, c):
            H1 = st.pop(("h1", c))
            rs = st.pop(("rs1", c))
            st[("r1", c)] = scale_relu(H1, rs, g1s, be1s, bigR1, "R1", "a")

        def s3_l2(st, c):
            """L2 + LN2 stats. For the simple-affine case, the LN2 rescale
            is per-column, so it commutes past relu AND the (linear) L3
            contraction: y = rs2 * (w3.T relu(H2)) + b3. Q2 = relu(H2) is
            produced here per m-tile; no broadcast matmul / pv2 needed."""
            R1 = st.pop(("r1", c))
            H2 = bigH.tile([128, MT * CH], F16, tag="H2")
            pst = ps_st.tile([1, CH], F32, tag="pst")
            tq_pairs = []
            npair = cfg["l2_pairs"]
            for mp in range(0, MT, npair):
                ms = range(mp, mp + npair)
                p2 = {}
                for m in ms:
                    p2m = ps_mm.tile([128, CH], F32, tag="pmm")
                    p2[m] = p2m
                for k in range(MT):
                    for m in ms:
                        nc.tensor.matmul(
                            p2[m][:], w2r[k][:, m * 128:(m + 1) * 128],
                            R1[:, k * CH:(k + 1) * CH],
                            start=(k == 0), stop=(k == MT - 1))
                for m in ms:
                    sl = slice(m * CH, (m + 1) * CH)
                    nc.scalar.activation(H2[:, sl], p2[m][:],
                                         AF.Identity, bias=bc2s[:, m:m + 1])
                    sq = sqp.tile([128, CH], F16, tag="sqb")
                    nc.vector.tensor_mul(sq[:], H2[:, sl], H2[:, sl])
                    if m % 2 == 0:
                        sq_hold = sq
                    else:
                        nc.vector.tensor_add(sq_hold[:], sq_hold[:], sq[:])
                        if (m // 2) % 2 == 0:
                            quad_hold = sq_hold
                        else:
                            nc.vector.tensor_add(quad_hold[:], quad_hold[:],
                                                 sq_hold[:])
                            nc.tensor.matmul(pst[:], ones_col[:],
                                             quad_hold[:],
                                             start=(m == 3),
                                             stop=(m == MT - 1))
                    if simple_affine:
                        # T_m = relu(H2_m)*w3_m, paired the same way:
                        # L3 contraction needs only 4 ones^T matmuls
                        tq = sqp.tile([128, CH], F16, tag="tq")
                        nc.vector.tensor_scalar(
                            out=tq[:], in0=H2[:, sl],
                            scalar1=0.0, scalar2=w3s32[:, m:m + 1],
                            op0=ALU.max, op1=ALU.mult)
                        if m % 2 == 0:
                            tq_hold = tq
                        else:
                            nc.vector.tensor_add(tq_hold[:], tq_hold[:], tq[:])
                            if (m // 2) % 2 == 0:
                                tq_quad = tq_hold
                            else:
                                nc.vector.tensor_add(tq_quad[:], tq_quad[:],
                                                     tq_hold[:])
                                tq_pairs.append(tq_quad)
            st[("h2", c)] = H2
            st[("tq", c)] = tq_pairs
            st[("rs2", c)] = rsqrt_stats(pst, "b", dt=F32)

        def s5_l3(st, c):
            H2 = st.pop(("h2", c))
            rs2 = st.pop(("rs2", c))
            tq_pairs = st.pop(("tq", c))
            p3 = ps_st.tile([1, CH], F32, tag="pst")
            if simple_affine:
                for j, tqp in enumerate(tq_pairs):
                    nc.tensor.matmul(p3[:], ones_col[:], tqp[:],
                                     start=(j == 0),
                                     stop=(j == len(tq_pairs) - 1))
            else:
                rs16 = small.tile([1, CH], F16, tag="rs2f16")
                nc.vector.tensor_copy(rs16[:], rs2[:])
                R2 = scale_relu(H2, rs16, g2s, be2s, bigR2, "R2x", "b")
                for k in range(MT):
                    nc.tensor.matmul(p3[:], w3p_r[:, k:k + 1],
                                     R2[:, k * CH:(k + 1) * CH],
                                     start=(k == 0), stop=(k == MT - 1))
            osb = small.tile([1, CH], F32, tag="osb")
            if simple_affine:
                yv = small.tile([1, CH], F32, tag="yv")
                nc.vector.tensor_mul(yv[:], p3[:], rs2[:])
                nc.scalar.activation(osb[:], yv[:], AF.Identity, bias=b3s[:])
            else:
                nc.scalar.activation(osb[:], p3[:], AF.Identity, bias=b3s[:])
            nc.sync.dma_start(y.ap()[0:1, c * CH:(c + 1) * CH], osb[:])

        def pipeline(st=None):
            if st is None:
                st = make_state()
                s0_xprep(st, 0)
            for i in range(NCH + 1):
                if i + 1 < NCH:
                    s0_xprep(st, i + 1)
                if i < NCH:
                    s1_l1(st, i)
                if 0 <= i - 1 < NCH:
                    s2_ln1(st, i - 1)
                    s3_l2(st, i - 1)
                    s5_l3(st, i - 1)

        if loop_iters is None:
            st0 = make_state()
            with tc.high_priority():
                s0_xprep(st0, 0)
            pipeline(st0)
        else:
            hint = tuple(
                mybir.EngineType(e) for e in
                ("PE", "Activation", "DVE", "SP", "Pool")
            ) if cfg.get("loop_hints") else ()
            with tc.For_i(0, loop_iters, 1,
                          staggered_reset=cfg.get("loop_stagger", False),
                          hint_engines=hint):
                pipeline()
    nc.compile()
    return nc


# ---------------------------------------------------------------- entry point
_CACHE = {}


BEST_CFG = dict(l2_pairs=4)


def _get_program(simple_affine):
    key = ("prog", simple_affine)
    if key not in _CACHE:
        _CACHE[key] = build_program(simple_affine, cfg=BEST_CFG)
    return _CACHE[key]


def make_in_maps(inputs):
    inp = {k: np.asarray(v) for k, v in inputs.items()}
    W1c, bc1, W2c, bc2 = _fold_weights(inp)
    XT = _build_xt(inp)
    g1 = np.asarray(inp["g1"], np.float32); be1 = np.asarray(inp["be1"], np.float32)
    g2 = np.asarray(inp["g2"], np.float32); be2 = np.asarray(inp["be2"], np.float32)
    simple_affine = bool(
        np.all(g1 == 1.0) and np.all(g2 == 1.0)
        and np.all(be1 == 0.0) and np.all(be2 == 0.0))
    W3 = np.asarray(inp["W3"], np.float32)
    b3 = np.asarray(inp["b3"], np.float32)
    shared = {
        "w1": W1c, "w2": W2c,
        "w3p": _pack128(W3[:, 0]).astype(np.float16),
        "bc1p": _pack128(bc1), "bc2p": _pack128(bc2),
        "g1p": _pack128(g1), "be1p": _pack128(be1),
        "g2p": _pack128(g2), "be2p": _pack128(be2),
        "b3t": b3.reshape(1, 1),
    }
    in_maps = []
    for c in range(N_CORES):
        m = dict(shared)
        m["xt"] = np.ascontiguousarray(XT[:, c * B_CORE:(c + 1) * B_CORE])
        in_maps.append(m)
    return in_maps, simple_affine


def kernel(**inputs) -> np.ndarray:
    in_maps, simple_affine = make_in_maps(inputs)
    nc = _get_program(simple_affine)
    res = run_bass_kernel_spmd(nc, in_maps, core_ids=list(range(N_CORES)))
    y = np.concatenate([r["y"][0] for r in res.results])
    return y.reshape(B, 1).astype(np.float32)


if __name__ == "__main__":
    import jax
    import reference
    cpu = jax.devices("cpu")[0]
    with jax.default_device(cpu):
        inp = reference.setup_inputs()
        ref = np.asarray(reference.reference(**inp))
    out = kernel(**{k: np.asarray(v) for k, v in inp.items()})
    err = np.abs(out - ref)
    scale = np.abs(ref).max()
    print("max_abs", err.max(), "rel(vs scale)", err.max() / scale,
          "mean_rel", (err / (np.abs(ref) + 1e-6)).mean())
